# revision 9
# baseline (speedup 1.0000x reference)
"""Trainium2 Bass kernel for Mllama-style GQA self-attention (B=1, S=2048,
H=32 q-heads, KVH=8 kv-heads, D=128), tensor-parallel over heads across 8
NeuronCores.

Sharding: core c owns kv-head c and q-heads 4c..4c+3 (Wq/Wk/Wv column shards),
computes its heads' attention output in transposed [feature, seq] layout,
AllGathers the 4096-feature activation across cores, then computes a
512-column shard of the output projection (Wo row shard). Host concatenates
the 8 output shards along the feature axis.

All matmuls run in bf16 (fp32 PSUM accumulation). Softmax skips the max
subtraction (scores are O(10) here so exp is safe in fp32) and normalizes
after the probs @ V matmul via a ones-row K=1 broadcast matmul; that
normalize chain is software-pipelined one attention unit behind the matmul
stream so the PE never waits on it.
"""
import math
import numpy as np
import ml_dtypes

import concourse.bass as bass
import concourse.bacc as bacc
import concourse.mybir as mybir
import concourse.tile as tile
from concourse.bass_utils import run_bass_kernel_spmd

BF16 = ml_dtypes.bfloat16
S, E, H, KVH, D = 2048, 4096, 32, 8, 128
N_CORES = 8
G = H // KVH                      # q heads per core
OSH = G * D                       # per-core q/attn feature count (512)
PC = 256                          # phase-1 seq chunk (projection rhs width)
AC = 512                          # attention sq chunk width
N_PC = S // PC                    # 8
N_AC = S // AC                    # 4
NE = E // 128                     # 32 contraction tiles
N_ST = S // 128                   # 16 seq tiles

_BUILD_CACHE = {}


def build_bass(causal: bool):
    key = causal
    if key in _BUILD_CACHE:
        return _BUILD_CACHE[key]
    dt = mybir.dt
    nc = bacc.Bacc("TRN2", target_bir_lowering=False, debug=False,
                   enable_asserts=False, num_devices=N_CORES)

    XT4 = nc.dram_tensor("xt", [N_PC, 128, NE, PC], dt.bfloat16, kind="ExternalInput").ap()
    WQT = nc.dram_tensor("wqt", [128, NE, OSH], dt.bfloat16, kind="ExternalInput").ap()
    WKT = nc.dram_tensor("wkt", [128, NE, D], dt.bfloat16, kind="ExternalInput").ap()
    WVT = nc.dram_tensor("wvt", [128, NE, D], dt.bfloat16, kind="ExternalInput").ap()
    WOT = nc.dram_tensor("wot", [128, NE, OSH], dt.bfloat16, kind="ExternalInput").ap()
    ROPE = nc.dram_tensor("rope", [4, D, S], dt.float32, kind="ExternalInput").ap()
    TRI = nc.dram_tensor("tri", [4, 128, AC], dt.bfloat16, kind="ExternalInput").ap()
    OUT = nc.dram_tensor("out", [S, OSH], dt.float32, kind="ExternalOutput").ap()

    with tile.TileContext(nc) as tc:
        with (
            tc.tile_pool(name="wpool", bufs=1) as wpool,
            tc.tile_pool(name="qkv", bufs=1) as qkvpool,
            tc.tile_pool(name="consts", bufs=1) as cpool,
            tc.tile_pool(name="xs", bufs=2) as xspool,
            tc.tile_pool(name="cs", bufs=2) as cspool,
            tc.tile_pool(name="rtmp", bufs=2) as rtmppool,
            tc.tile_pool(name="epool", bufs=3) as epool,
            tc.tile_pool(name="small", bufs=2) as smallpool,
            tc.tile_pool(name="attn", bufs=2) as attnpool,
            tc.tile_pool(name="agp", bufs=1) as agpool,
            tc.tile_pool(name="outs", bufs=2) as outpool,
            tc.tile_pool(name="ps_qkv", bufs=2, space="PSUM") as ps_qkv,
            tc.tile_pool(name="ps_s", bufs=2, space="PSUM") as ps_s,
            tc.tile_pool(name="ps_ot", bufs=3, space="PSUM") as ps_ot,
            tc.tile_pool(name="ps_den", bufs=1, space="PSUM") as ps_den,
            tc.tile_pool(name="dram", bufs=3, space="DRAM") as drampool,
        ):
            # --- resident weights, [128, NE, width]. Priority order: the
            # first chunk's activations and Wk jump ahead of the big weight
            # burst so the PE starts in ~10us instead of ~70.
            wq_sb = wpool.tile([128, NE, OSH], dt.bfloat16)
            wk_sb = wpool.tile([128, NE, D], dt.bfloat16)
            wv_sb = wpool.tile([128, NE, D], dt.bfloat16)
            wo_sb = wpool.tile([128, NE, OSH], dt.bfloat16)

            xs0 = xspool.tile([128, NE, PC], dt.bfloat16, tag="xs")
            cs0 = cspool.tile([128, 4, PC], dt.float32, tag="cs")
            for q in range(8):
                nc.sync.dma_start(xs0[:, q * 4:(q + 1) * 4, :],
                                  XT4[0, :, q * 4:(q + 1) * 4, :])
            for q in range(8):
                nc.sync.dma_start(wk_sb[:, q * 4:(q + 1) * 4, :],
                                  WKT[:, q * 4:(q + 1) * 4, :])
            for q in range(8):
                nc.sync.dma_start(wv_sb[:, q * 4:(q + 1) * 4, :],
                                  WVT[:, q * 4:(q + 1) * 4, :])
            nc.sync.dma_start(cs0[:], ROPE[:, :, 0:PC].rearrange("j p s -> p j s"))
            for q in range(8):
                nc.sync.dma_start(wq_sb[:, q * 4:(q + 1) * 4, :],
                                  WQT[:, q * 4:(q + 1) * 4, :])

            tri_sb = cpool.tile([128, 4, AC], dt.bfloat16)
            nc.sync.dma_start(tri_sb[:], TRI.rearrange("j p f -> p j f"))
            ones_col = cpool.tile([128, 1], dt.bfloat16)
            nc.vector.memset(ones_col[:], 1.0)
            ones_row = cpool.tile([1, 128], dt.bfloat16)
            nc.vector.memset(ones_row[:], 1.0)
            # tiny AllGather to absorb first-collective latency, overlapped
            # with phase 1 (collectives run on TOPSP/SDMA, not the engines)
            warm_in = drampool.tile([128, 1], dt.bfloat16, tag="warm_in")
            nc.sync.dma_start(warm_in[:], ones_col[:])
            warm_out = drampool.tile([128 * N_CORES, 1], dt.bfloat16,
                                     tag="warm_out", addr_space="Shared")
            nc.gpsimd.collective_compute(
                "AllGather", mybir.AluOpType.bypass,
                replica_groups=[list(range(N_CORES))],
                ins=[warm_in.opt()], outs=[warm_out.opt()])

            # --- persistent activations
            qT_sb = qkvpool.tile([128, G, S], dt.bfloat16)     # per-head [d, s]
            kT_sb = qkvpool.tile([128, S], dt.bfloat16)        # [d, s]
            v_sb = qkvpool.tile([128, N_ST, D], dt.bfloat16)   # per s-tile [t, d]

            # ================= Phase 1: QKV projections + RoPE ==============
            for sc in range(N_PC):
                s0 = sc * PC
                if sc == 0:
                    xs, cs = xs0, cs0
                else:
                    xs = xspool.tile([128, NE, PC], dt.bfloat16, tag="xs")
                    for q in range(4):
                        nc.sync.dma_start(xs[:, q * 8:(q + 1) * 8, :],
                                          XT4[sc, :, q * 8:(q + 1) * 8, :])
                    cs = cspool.tile([128, 4, PC], dt.float32, tag="cs")
                    nc.sync.dma_start(cs[:], ROPE[:, :, s0:s0 + PC]
                                      .rearrange("j p s -> p j s"))
                cosq_t, sinq_t = cs[:, 0, :], cs[:, 1, :]
                cosk_t, sink_t = cs[:, 2, :], cs[:, 3, :]

                # k head first, then v, then q heads (k/v weights land first)
                for hh in [G, -1] + list(range(G)):
                    if hh == -1:
                        for u in range(PC // 128):
                            st = (s0 // 128) + u
                            pv = ps_qkv.tile([128, D], dt.float32, tag="pq")
                            for e in range(NE):
                                nc.tensor.matmul(pv[:],
                                                 xs[:, e, u * 128:(u + 1) * 128],
                                                 wv_sb[:, e, :],
                                                 start=(e == 0), stop=(e == NE - 1))
                            nc.vector.tensor_copy(v_sb[:, st, :], pv[:])
                        continue
                    is_k = hh == G
                    pq = ps_qkv.tile([128, PC], dt.float32, tag="pq")
                    for e in range(NE):
                        lhsT = (wk_sb[:, e, :] if is_k
                                else wq_sb[:, e, hh * D:(hh + 1) * D])
                        nc.tensor.matmul(pq[:], lhsT, xs[:, e, :],
                                         start=(e == 0), stop=(e == NE - 1))
                    cos_t, sin_t = (cosk_t, sink_t) if is_k else (cosq_t, sinq_t)
                    dest = kT_sb[:, s0:s0 + PC] if is_k \
                        else qT_sb[:, hh, s0:s0 + PC]
                    t1 = rtmppool.tile([128, PC], dt.float32, tag="t1")
                    t2 = rtmppool.tile([128, PC], dt.float32, tag="t2")
                    # low half: q'= q_lo*cos_lo + q_hi*sin_mod_lo
                    nc.vector.tensor_mul(t1[0:64, :], pq[0:64, :], cos_t[0:64, :])
                    nc.vector.tensor_mul(t2[0:64, :], pq[64:128, :], sin_t[0:64, :])
                    nc.vector.tensor_add(dest[0:64, :], t1[0:64, :], t2[0:64, :])
                    # high half: q'= q_hi*cos_hi + q_lo*sin_mod_hi
                    nc.vector.tensor_mul(t1[64:128, :], pq[64:128, :], cos_t[64:128, :])
                    nc.vector.tensor_mul(t2[64:128, :], pq[0:64, :], sin_t[64:128, :])
                    nc.vector.tensor_add(dest[64:128, :], t1[64:128, :], t2[64:128, :])


            # Wo shard load deferred to here so it doesn't fight startup DMAs
            for q in range(2):
                nc.sync.dma_start(wo_sb[:, q * 16:(q + 1) * 16, :],
                                  WOT[:, q * 16:(q + 1) * 16, :])

            # ============ Phase 2: attention, one (sq-chunk, head) unit at a
            # time; each unit's normalize/store tail is emitted after the NEXT
            # unit's matmuls so the PE stream never waits on DVE/ACT work.
            # Phase 3 (o_proj) for chunk j-1 is emitted after chunk j's units,
            # by which time its AllGather has had a full chunk to complete.
            ag_ins = {}
            ag_outs = {}
            pending = []            # [(j, h, ot, esum), ...] depth-2 pipeline

            def emit_tail(u):
                j, h, ot, esum = u
                den = ps_den.tile([1, AC], dt.float32, tag="den")
                nc.tensor.matmul(den[:], ones_col[:], esum[:],
                                 start=True, stop=True)
                recip = smallpool.tile([1, AC], dt.bfloat16, tag="recip")
                with nc.allow_low_precision(reason="softmax denom recip bf16"):
                    nc.vector.reciprocal(recip[:], den[:])
                bc_ps = ps_s.tile([128, AC], dt.float32, tag="st")
                nc.tensor.matmul(bc_ps[:], ones_row[:], recip[:],
                                 start=True, stop=True)
                bc_sb = smallpool.tile([128, AC], dt.bfloat16, tag="bc")
                nc.scalar.copy(bc_sb[:], bc_ps[:])
                attnT = attnpool.tile([128, AC], dt.bfloat16, tag="attnT")
                nc.vector.tensor_mul(attnT[:], ot[:], bc_sb[:])
                nc.sync.dma_start(ag_ins[j][h * 128:(h + 1) * 128, :], attnT[:])
                if h == G - 1:
                    ag_out = drampool.tile([E, AC], dt.bfloat16, tag="ag_out",
                                           addr_space="Shared")
                    nc.gpsimd.collective_compute(
                        "AllGather", mybir.AluOpType.bypass,
                        replica_groups=[list(range(N_CORES))],
                        ins=[ag_ins[j].opt()], outs=[ag_out.opt()])
                    ag_outs[j] = ag_out

            def emit_phase3(j):
                ag_out = ag_outs[j]
                ag_sb = agpool.tile([128, NE, AC], dt.bfloat16, tag="ag")
                for eg in range(8):
                    nc.sync.dma_start(
                        ag_sb[:, eg * 4:(eg + 1) * 4, :],
                        ag_out[eg * 512:(eg + 1) * 512, :]
                        .rearrange("(n p) s -> p n s", p=128))
                for t in range(AC // 128):
                    po = ps_qkv.tile([128, OSH], dt.float32, tag="pq")
                    for e in range(NE):
                        nc.tensor.matmul(po[:], ag_sb[:, e, t * 128:(t + 1) * 128],
                                         wo_sb[:, e, :],
                                         start=(e == 0), stop=(e == NE - 1))
                    o_sb = outpool.tile([128, OSH], dt.float32, tag="o")
                    nc.vector.tensor_copy(o_sb[:], po[:])
                    nc.sync.dma_start(
                        OUT[j * AC + t * 128: j * AC + (t + 1) * 128, :], o_sb[:])

            for j in range(N_AC):
                c0 = j * AC
                nb = (4 * (j + 1)) if causal else N_ST
                ag_ins[j] = drampool.tile([OSH, AC], dt.bfloat16, tag="ag_in",
                                          name=f"ag_in{j}")
                for h in range(G):
                    ot = ps_ot.tile([128, AC], dt.float32, tag="ot")
                    esum = epool.tile([128, AC], dt.bfloat16, tag="esum")
                    for b in range(nb):
                        st_ps = ps_s.tile([128, AC], dt.float32, tag="st")
                        nc.tensor.matmul(st_ps[:], kT_sb[:, b * 128:(b + 1) * 128],
                                         qT_sb[:, h, c0:c0 + AC],
                                         start=True, stop=True)
                        e_sb = epool.tile([128, AC], dt.bfloat16, tag="e")
                        nc.scalar.activation(e_sb[:], st_ps[:],
                                             mybir.ActivationFunctionType.Exp)
                        if causal and b >= 4 * j:
                            nc.vector.tensor_mul(e_sb[:], e_sb[:],
                                                 tri_sb[:, b - 4 * j, :])
                        nc.tensor.matmul(ot[:], v_sb[:, b, :], e_sb[:],
                                         start=(b == 0), stop=(b == nb - 1))
                        with nc.allow_low_precision(reason="softmax denom bf16"):
                            if b == 0:
                                nc.vector.tensor_copy(esum[:], e_sb[:])
                            else:
                                nc.vector.tensor_add(esum[:], esum[:], e_sb[:])
                    pending.append((j, h, ot, esum))
                    if len(pending) > 2:
                        emit_tail(pending.pop(0))
                if j >= 2:
                    emit_phase3(j - 2)

            for u in pending:
                emit_tail(u)
            emit_phase3(N_AC - 2)
            emit_phase3(N_AC - 1)

    nc.compile()
    _BUILD_CACHE[key] = nc
    return nc


def _prep_inputs(hidden_states, attention_mask, cos, sin, Wq, Wk, Wv, Wo):
    X = np.asarray(hidden_states, dtype=np.float32).reshape(S, E)
    # [N_PC, 128, NE, PC]: exact SBUF tile layout per chunk -> long DMA runs
    XT4 = np.ascontiguousarray(
        X.reshape(N_PC, PC, NE, 128).transpose(0, 3, 2, 1)).astype(BF16)

    m = np.asarray(attention_mask, dtype=np.float32).reshape(S, S)
    il, ju = np.tril_indices(S), np.triu_indices(S, 1)
    causal = bool(np.all(m[il] == 0.0) and np.all(m[ju] <= -1e8))
    dense = bool(np.all(m == 0.0))
    if not (causal or dense):
        raise NotImplementedError("only causal or all-zero masks supported")

    scale = 1.0 / math.sqrt(D)
    cosT = np.ascontiguousarray(np.asarray(cos, np.float32).reshape(S, D).T)
    sinT = np.ascontiguousarray(np.asarray(sin, np.float32).reshape(S, D).T)
    sin_mod = sinT.copy()
    sin_mod[0:64] *= -1.0
    rope_t = np.stack([cosT * scale, sin_mod * scale, cosT, sin_mod]) \
        .astype(np.float32)

    p = np.arange(128)[:, None]
    f = np.arange(AC)[None, :]
    tri = np.stack([(128 * jj + p <= f) for jj in range(4)]).astype(BF16)

    Wq = np.asarray(Wq, np.float32)
    Wk = np.asarray(Wk, np.float32)
    Wv = np.asarray(Wv, np.float32)
    Wo = np.asarray(Wo, np.float32)

    def wtile(Wshard):
        # [out, E] -> SBUF layout [128, NE, out]
        return np.ascontiguousarray(
            Wshard.T.reshape(NE, 128, Wshard.shape[0]).transpose(1, 0, 2)
        ).astype(BF16)

    in_maps = []
    for c in range(N_CORES):
        in_maps.append({
            "xt": XT4,
            "wqt": wtile(Wq[c * OSH:(c + 1) * OSH, :]),
            "wkt": wtile(Wk[c * D:(c + 1) * D, :]),
            "wvt": wtile(Wv[c * D:(c + 1) * D, :]),
            "wot": wtile(Wo[c * OSH:(c + 1) * OSH, :]),
            "rope": rope_t,
            "tri": tri,
        })
    return in_maps, causal


def kernel(hidden_states, attention_mask, cos, sin, Wq, Wk, Wv, Wo,
           _trace=False, _tmpdir=None):
    in_maps, causal = _prep_inputs(hidden_states, attention_mask, cos, sin,
                                   Wq, Wk, Wv, Wo)
    nc = build_bass(causal)
    res = run_bass_kernel_spmd(nc, in_maps, core_ids=list(range(N_CORES)),
                               trace=_trace, tmpdir=_tmpdir)
    out = np.concatenate([res.results[c]["out"] for c in range(N_CORES)], axis=1)
    kernel._last_result = res
    return out.reshape(1, S, E).astype(np.float32)


# revision 12
# speedup vs baseline: 1.0781x; 1.0781x over previous
"""Trainium2 Bass kernel for Mllama-style GQA self-attention (B=1, S=2048,
H=32 q-heads, KVH=8 kv-heads, D=128), tensor-parallel over heads across 8
NeuronCores.

Sharding: core c owns kv-head c and q-heads 4c..4c+3 (Wq/Wk/Wv column shards),
computes its heads' attention output in transposed [feature, seq] layout,
AllGathers the 4096-feature activation across cores, then computes a
512-column shard of the output projection (Wo row shard). Host concatenates
the 8 output shards along the feature axis.

All matmuls run in bf16 (fp32 PSUM accumulation). Softmax skips the max
subtraction (scores are O(10) here so exp is safe in fp32) and normalizes
after the probs @ V matmul via a ones-row K=1 broadcast matmul; that
normalize chain is software-pipelined one attention unit behind the matmul
stream so the PE never waits on it.
"""
import math
import numpy as np
import ml_dtypes

import concourse.bass as bass
import concourse.bacc as bacc
import concourse.mybir as mybir
import concourse.tile as tile
from concourse.bass_utils import run_bass_kernel_spmd

BF16 = ml_dtypes.bfloat16
S, E, H, KVH, D = 2048, 4096, 32, 8, 128
N_CORES = 8
G = H // KVH                      # q heads per core
OSH = G * D                       # per-core q/attn feature count (512)
PC = 256                          # phase-1 seq chunk (projection rhs width)
AC = 512                          # attention sq chunk width
N_PC = S // PC                    # 8
N_AC = S // AC                    # 4
NE = E // 128                     # 32 contraction tiles
N_ST = S // 128                   # 16 seq tiles

_BUILD_CACHE = {}


def build_bass(causal: bool):
    key = causal
    if key in _BUILD_CACHE:
        return _BUILD_CACHE[key]
    dt = mybir.dt
    nc = bacc.Bacc("TRN2", target_bir_lowering=False, debug=False,
                   enable_asserts=False, num_devices=N_CORES)

    XT4 = nc.dram_tensor("xt", [N_PC, 128, NE, PC], dt.bfloat16, kind="ExternalInput").ap()
    WQT = nc.dram_tensor("wqt", [128, NE, OSH], dt.bfloat16, kind="ExternalInput").ap()
    WKT = nc.dram_tensor("wkt", [128, NE, D], dt.bfloat16, kind="ExternalInput").ap()
    WVT = nc.dram_tensor("wvt", [128, NE, D], dt.bfloat16, kind="ExternalInput").ap()
    WOT = nc.dram_tensor("wot", [128, NE, OSH], dt.bfloat16, kind="ExternalInput").ap()
    ROPE = nc.dram_tensor("rope", [4, D, S], dt.float32, kind="ExternalInput").ap()
    TRI = nc.dram_tensor("tri", [4, 128, AC], dt.bfloat16, kind="ExternalInput").ap()
    OUT = nc.dram_tensor("out", [S, OSH], dt.float32, kind="ExternalOutput").ap()

    with tile.TileContext(nc) as tc:
        with (
            tc.tile_pool(name="wpool", bufs=1) as wpool,
            tc.tile_pool(name="qkv", bufs=1) as qkvpool,
            tc.tile_pool(name="consts", bufs=1) as cpool,
            tc.tile_pool(name="xs", bufs=2) as xspool,
            tc.tile_pool(name="cs", bufs=2) as cspool,
            tc.tile_pool(name="rtmp", bufs=2) as rtmppool,
            tc.tile_pool(name="epool", bufs=3) as epool,
            tc.tile_pool(name="small", bufs=2) as smallpool,
            tc.tile_pool(name="attn", bufs=2) as attnpool,
            tc.tile_pool(name="agp", bufs=2) as agpool,
            tc.tile_pool(name="outs", bufs=2) as outpool,
            tc.tile_pool(name="ps_qkv", bufs=2, space="PSUM") as ps_qkv,
            tc.tile_pool(name="ps_s", bufs=2, space="PSUM") as ps_s,
            tc.tile_pool(name="ps_ot", bufs=3, space="PSUM") as ps_ot,
            tc.tile_pool(name="ps_den", bufs=1, space="PSUM") as ps_den,
            tc.tile_pool(name="dram", bufs=3, space="DRAM") as drampool,
        ):
            # --- resident weights, [128, NE, width]. Priority order: the
            # first chunk's activations and Wk jump ahead of the big weight
            # burst so the PE starts in ~10us instead of ~70.
            wq_sb = wpool.tile([128, NE, OSH], dt.bfloat16)
            wk_sb = wpool.tile([128, NE, D], dt.bfloat16)
            wv_sb = wpool.tile([128, NE, D], dt.bfloat16)
            wo_sb = wpool.tile([128, NE, OSH], dt.bfloat16)

            xs0 = xspool.tile([128, NE, PC], dt.bfloat16, tag="xs")
            cs0 = cspool.tile([128, 4, PC], dt.float32, tag="cs")
            for q in range(8):
                nc.sync.dma_start(xs0[:, q * 4:(q + 1) * 4, :],
                                  XT4[0, :, q * 4:(q + 1) * 4, :])
            for q in range(8):
                nc.sync.dma_start(wk_sb[:, q * 4:(q + 1) * 4, :],
                                  WKT[:, q * 4:(q + 1) * 4, :])
            for q in range(8):
                nc.sync.dma_start(wv_sb[:, q * 4:(q + 1) * 4, :],
                                  WVT[:, q * 4:(q + 1) * 4, :])
            nc.sync.dma_start(cs0[:], ROPE[:, :, 0:PC].rearrange("j p s -> p j s"))
            for q in range(8):
                nc.sync.dma_start(wq_sb[:, q * 4:(q + 1) * 4, :],
                                  WQT[:, q * 4:(q + 1) * 4, :])

            tri_sb = cpool.tile([128, 4, AC], dt.bfloat16)
            nc.sync.dma_start(tri_sb[:], TRI.rearrange("j p f -> p j f"))
            ones_col = cpool.tile([128, 1], dt.bfloat16)
            nc.vector.memset(ones_col[:], 1.0)
            ones_row = cpool.tile([1, 128], dt.bfloat16)
            nc.vector.memset(ones_row[:], 1.0)
            # tiny AllGather to absorb first-collective latency, overlapped
            # with phase 1 (collectives run on TOPSP/SDMA, not the engines)
            warm_in = drampool.tile([128, 1], dt.bfloat16, tag="warm_in")
            nc.sync.dma_start(warm_in[:], ones_col[:])
            warm_out = drampool.tile([128 * N_CORES, 1], dt.bfloat16,
                                     tag="warm_out", addr_space="Shared")
            nc.gpsimd.collective_compute(
                "AllGather", mybir.AluOpType.bypass,
                replica_groups=[list(range(N_CORES))],
                ins=[warm_in.opt()], outs=[warm_out.opt()])

            # --- persistent activations
            qT_sb = qkvpool.tile([128, G, S], dt.bfloat16)     # per-head [d, s]
            kT_sb = qkvpool.tile([128, S], dt.bfloat16)        # [d, s]
            v_sb = qkvpool.tile([128, N_ST, D], dt.bfloat16)   # per s-tile [t, d]

            # ================= Phase 1: QKV projections + RoPE ==============
            for sc in range(N_PC):
                s0 = sc * PC
                if sc == 0:
                    xs, cs = xs0, cs0
                else:
                    xs = xspool.tile([128, NE, PC], dt.bfloat16, tag="xs")
                    for q in range(4):
                        nc.sync.dma_start(xs[:, q * 8:(q + 1) * 8, :],
                                          XT4[sc, :, q * 8:(q + 1) * 8, :])
                    cs = cspool.tile([128, 4, PC], dt.float32, tag="cs")
                    nc.sync.dma_start(cs[:], ROPE[:, :, s0:s0 + PC]
                                      .rearrange("j p s -> p j s"))
                cosq_t, sinq_t = cs[:, 0, :], cs[:, 1, :]
                cosk_t, sink_t = cs[:, 2, :], cs[:, 3, :]

                # k head first, then v, then q heads (k/v weights land first)
                for hh in [G, -1] + list(range(G)):
                    if hh == -1:
                        for u in range(PC // 128):
                            st = (s0 // 128) + u
                            pv = ps_qkv.tile([128, D], dt.float32, tag="pq")
                            for e in range(NE):
                                nc.tensor.matmul(pv[:],
                                                 xs[:, e, u * 128:(u + 1) * 128],
                                                 wv_sb[:, e, :],
                                                 start=(e == 0), stop=(e == NE - 1))
                            nc.vector.tensor_copy(v_sb[:, st, :], pv[:])
                        continue
                    is_k = hh == G
                    pq = ps_qkv.tile([128, PC], dt.float32, tag="pq")
                    for e in range(NE):
                        lhsT = (wk_sb[:, e, :] if is_k
                                else wq_sb[:, e, hh * D:(hh + 1) * D])
                        nc.tensor.matmul(pq[:], lhsT, xs[:, e, :],
                                         start=(e == 0), stop=(e == NE - 1))
                    cos_t, sin_t = (cosk_t, sink_t) if is_k else (cosq_t, sinq_t)
                    dest = kT_sb[:, s0:s0 + PC] if is_k \
                        else qT_sb[:, hh, s0:s0 + PC]
                    t1 = rtmppool.tile([128, PC], dt.float32, tag="t1")
                    t2 = rtmppool.tile([128, PC], dt.float32, tag="t2")
                    # low half: q'= q_lo*cos_lo + q_hi*sin_mod_lo
                    nc.vector.tensor_mul(t1[0:64, :], pq[0:64, :], cos_t[0:64, :])
                    nc.vector.tensor_mul(t2[0:64, :], pq[64:128, :], sin_t[0:64, :])
                    nc.vector.tensor_add(dest[0:64, :], t1[0:64, :], t2[0:64, :])
                    # high half: q'= q_hi*cos_hi + q_lo*sin_mod_hi
                    nc.vector.tensor_mul(t1[64:128, :], pq[64:128, :], cos_t[64:128, :])
                    nc.vector.tensor_mul(t2[64:128, :], pq[0:64, :], sin_t[64:128, :])
                    nc.vector.tensor_add(dest[64:128, :], t1[64:128, :], t2[64:128, :])


            # Wo shard load deferred to here so it doesn't fight startup DMAs
            for q in range(2):
                nc.sync.dma_start(wo_sb[:, q * 16:(q + 1) * 16, :],
                                  WOT[:, q * 16:(q + 1) * 16, :])

            # ============ Phase 2: attention as (col-range, head) units with a
            # depth-2 software-pipelined normalize/store tail (PE never waits
            # on the DVE/ACT chain). Chunks 0 and 1 are interleaved so tiny
            # early units still give the tail chain enough runway; the last
            # chunk is split into two half-width AllGathers so the final
            # o_proj pipelines against the last collective instead of
            # serializing behind it.
            # Unit descriptor: (key, c0, cw, nb, diag0, h)
            ag_ins = {}
            ag_outs = {}
            ag_meta = {}            # key -> (c0, cw)
            pending = []

            def make_units():
                units, keys = [], []
                if not causal:
                    for j in range(N_AC):
                        key = str(j)
                        keys.append((key, j * AC, AC, N_ST, N_ST))
                        for h in range(G):
                            units.append((key, j * AC, AC, N_ST, N_ST, h))
                    return units, keys
                # chunks 0,1 interleaved
                for j in (0, 1):
                    keys.append((str(j), j * AC, AC, 4 * (j + 1), 4 * j))
                for h in range(G):
                    for j in (0, 1):
                        units.append((str(j), j * AC, AC, 4 * (j + 1), 4 * j, h))
                keys.append(("2", 2 * AC, AC, 12, 8))
                units += [("2", 2 * AC, AC, 12, 8, h) for h in range(G)]
                for half in (0, 1):
                    c0 = 3 * AC + half * (AC // 2)
                    nb = 14 + 2 * half
                    key = "3a" if half == 0 else "3b"
                    keys.append((key, c0, AC // 2, nb, nb - 2))
                    units += [(key, c0, AC // 2, nb, nb - 2, h) for h in range(G)]
                return units, keys

            units, keys = make_units()
            remaining = {k[0]: G for k in keys}
            for key, c0, cw, nb, d0 in keys:
                ag_meta[key] = (c0, cw)
                ag_ins[key] = drampool.tile([OSH, cw], dt.bfloat16, tag="ag_in",
                                            name=f"ag_in{key}")

            def emit_tail(u):
                key, c0, cw, nb, d0, h, ot, esum = u
                den = ps_den.tile([1, cw], dt.float32, tag="den")
                nc.tensor.matmul(den[:], ones_col[:], esum[:],
                                 start=True, stop=True)
                recip = smallpool.tile([1, cw], dt.bfloat16, tag="recip")
                with nc.allow_low_precision(reason="softmax denom recip bf16"):
                    nc.vector.reciprocal(recip[:], den[:])
                bc_ps = ps_s.tile([128, cw], dt.float32, tag="st")
                nc.tensor.matmul(bc_ps[:], ones_row[:], recip[:],
                                 start=True, stop=True)
                bc_sb = smallpool.tile([128, cw], dt.bfloat16, tag="bc")
                nc.scalar.copy(bc_sb[:], bc_ps[:])
                attnT = attnpool.tile([128, cw], dt.bfloat16, tag="attnT")
                nc.vector.tensor_mul(attnT[:], ot[:], bc_sb[:])
                nc.sync.dma_start(ag_ins[key][h * 128:(h + 1) * 128, :], attnT[:])
                remaining[key] -= 1
                if remaining[key] == 0:
                    ag_out = drampool.tile([E, cw], dt.bfloat16, tag="ag_out",
                                           addr_space="Shared",
                                           name=f"ag_out{key}")
                    nc.gpsimd.collective_compute(
                        "AllGather", mybir.AluOpType.bypass,
                        replica_groups=[list(range(N_CORES))],
                        ins=[ag_ins[key].opt()], outs=[ag_out.opt()])
                    ag_outs[key] = ag_out

            def emit_phase3(key):
                ag_out = ag_outs[key]
                c0, cw = ag_meta[key]
                for hf in range(cw // 256):
                    f0 = hf * 256
                    ag_sb = agpool.tile([128, NE, 256], dt.bfloat16, tag="ag")
                    for eg in range(8):
                        nc.sync.dma_start(
                            ag_sb[:, eg * 4:(eg + 1) * 4, :],
                            ag_out[eg * 512:(eg + 1) * 512, f0:f0 + 256]
                            .rearrange("(n p) s -> p n s", p=128))
                    for t in range(2):
                        po = ps_qkv.tile([128, OSH], dt.float32, tag="pq")
                        for e in range(NE):
                            nc.tensor.matmul(
                                po[:], ag_sb[:, e, t * 128:(t + 1) * 128],
                                wo_sb[:, e, :],
                                start=(e == 0), stop=(e == NE - 1))
                        o_sb = outpool.tile([128, OSH], dt.float32, tag="o")
                        nc.vector.tensor_copy(o_sb[:], po[:])
                        nc.sync.dma_start(
                            OUT[c0 + f0 + t * 128: c0 + f0 + (t + 1) * 128, :],
                            o_sb[:])

            phase3_after = {}        # emit phase3(key) after unit index i
            if causal:
                phase3_after = {11: "0", 15: "1"}   # after chunk 2 / chunk 3a
            else:
                phase3_after = {11: "0", 15: "1"}

            for i, (key, c0, cw, nb, d0, h) in enumerate(units):
                ot = ps_ot.tile([128, cw], dt.float32, tag="ot")
                esum = epool.tile([128, cw], dt.bfloat16, tag="esum")
                for b in range(nb):
                    st_ps = ps_s.tile([128, cw], dt.float32, tag="st")
                    nc.tensor.matmul(st_ps[:], kT_sb[:, b * 128:(b + 1) * 128],
                                     qT_sb[:, h, c0:c0 + cw],
                                     start=True, stop=True)
                    e_sb = epool.tile([128, cw], dt.bfloat16, tag="e")
                    nc.scalar.activation(e_sb[:], st_ps[:],
                                         mybir.ActivationFunctionType.Exp)
                    if causal and b >= d0:
                        nc.vector.tensor_mul(e_sb[:], e_sb[:],
                                             tri_sb[:, b - d0, 0:cw])
                    nc.tensor.matmul(ot[:], v_sb[:, b, :], e_sb[:],
                                     start=(b == 0), stop=(b == nb - 1))
                    with nc.allow_low_precision(reason="softmax denom bf16"):
                        if b == 0:
                            nc.vector.tensor_copy(esum[:], e_sb[:])
                        else:
                            nc.vector.tensor_add(esum[:], esum[:], e_sb[:])
                pending.append((key, c0, cw, nb, d0, h, ot, esum))
                if len(pending) > 2:
                    emit_tail(pending.pop(0))
                if i in phase3_after:
                    emit_phase3(phase3_after[i])

            for u in pending:
                emit_tail(u)
            if causal:
                for key in ("2", "3a", "3b"):
                    emit_phase3(key)
            else:
                for key in ("2", "3"):
                    emit_phase3(key)

    nc.compile()
    _BUILD_CACHE[key] = nc
    return nc


def _prep_inputs(hidden_states, attention_mask, cos, sin, Wq, Wk, Wv, Wo):
    X = np.asarray(hidden_states, dtype=np.float32).reshape(S, E)
    # [N_PC, 128, NE, PC]: exact SBUF tile layout per chunk -> long DMA runs
    XT4 = np.ascontiguousarray(
        X.reshape(N_PC, PC, NE, 128).transpose(0, 3, 2, 1)).astype(BF16)

    m = np.asarray(attention_mask, dtype=np.float32).reshape(S, S)
    il, ju = np.tril_indices(S), np.triu_indices(S, 1)
    causal = bool(np.all(m[il] == 0.0) and np.all(m[ju] <= -1e8))
    dense = bool(np.all(m == 0.0))
    if not (causal or dense):
        raise NotImplementedError("only causal or all-zero masks supported")

    scale = 1.0 / math.sqrt(D)
    cosT = np.ascontiguousarray(np.asarray(cos, np.float32).reshape(S, D).T)
    sinT = np.ascontiguousarray(np.asarray(sin, np.float32).reshape(S, D).T)
    sin_mod = sinT.copy()
    sin_mod[0:64] *= -1.0
    rope_t = np.stack([cosT * scale, sin_mod * scale, cosT, sin_mod]) \
        .astype(np.float32)

    p = np.arange(128)[:, None]
    f = np.arange(AC)[None, :]
    tri = np.stack([(128 * jj + p <= f) for jj in range(4)]).astype(BF16)

    Wq = np.asarray(Wq, np.float32)
    Wk = np.asarray(Wk, np.float32)
    Wv = np.asarray(Wv, np.float32)
    Wo = np.asarray(Wo, np.float32)

    def wtile(Wshard):
        # [out, E] -> SBUF layout [128, NE, out]
        return np.ascontiguousarray(
            Wshard.T.reshape(NE, 128, Wshard.shape[0]).transpose(1, 0, 2)
        ).astype(BF16)

    in_maps = []
    for c in range(N_CORES):
        in_maps.append({
            "xt": XT4,
            "wqt": wtile(Wq[c * OSH:(c + 1) * OSH, :]),
            "wkt": wtile(Wk[c * D:(c + 1) * D, :]),
            "wvt": wtile(Wv[c * D:(c + 1) * D, :]),
            "wot": wtile(Wo[c * OSH:(c + 1) * OSH, :]),
            "rope": rope_t,
            "tri": tri,
        })
    return in_maps, causal


def kernel(hidden_states, attention_mask, cos, sin, Wq, Wk, Wv, Wo,
           _trace=False, _tmpdir=None):
    in_maps, causal = _prep_inputs(hidden_states, attention_mask, cos, sin,
                                   Wq, Wk, Wv, Wo)
    nc = build_bass(causal)
    res = run_bass_kernel_spmd(nc, in_maps, core_ids=list(range(N_CORES)),
                               trace=_trace, tmpdir=_tmpdir)
    out = np.concatenate([res.results[c]["out"] for c in range(N_CORES)], axis=1)
    kernel._last_result = res
    return out.reshape(1, S, E).astype(np.float32)


# revision 15
# speedup vs baseline: 1.0858x; 1.0072x over previous
"""Trainium2 Bass kernel for Mllama-style GQA self-attention (B=1, S=2048,
H=32 q-heads, KVH=8 kv-heads, D=128), tensor-parallel over heads across 8
NeuronCores.

Sharding: core c owns kv-head c and q-heads 4c..4c+3 (Wq/Wk/Wv column shards),
computes its heads' attention output in transposed [feature, seq] layout,
AllGathers the 4096-feature activation across cores, then computes a
512-column shard of the output projection (Wo row shard). Host concatenates
the 8 output shards along the feature axis.

All matmuls run in bf16 (fp32 PSUM accumulation). Softmax skips the max
subtraction (scores are O(10) here so exp is safe in fp32) and normalizes
after the probs @ V matmul via a ones-row K=1 broadcast matmul; that
normalize chain is software-pipelined one attention unit behind the matmul
stream so the PE never waits on it.
"""
import math
from contextlib import ExitStack
import numpy as np
import ml_dtypes

import concourse.bass as bass
import concourse.bacc as bacc
import concourse.mybir as mybir
import concourse.tile as tile
from concourse.bass_utils import run_bass_kernel_spmd

BF16 = ml_dtypes.bfloat16
S, E, H, KVH, D = 2048, 4096, 32, 8, 128
N_CORES = 8
G = H // KVH                      # q heads per core
OSH = G * D                       # per-core q/attn feature count (512)
PC = 512                          # phase-1 seq chunk (projection rhs width)
AC = 512                          # attention sq chunk width
N_PC = S // PC                    # 8
N_AC = S // AC                    # 4
NE = E // 128                     # 32 contraction tiles
N_ST = S // 128                   # 16 seq tiles

_BUILD_CACHE = {}


def build_bass(causal: bool):
    key = causal
    if key in _BUILD_CACHE:
        return _BUILD_CACHE[key]
    dt = mybir.dt
    nc = bacc.Bacc("TRN2", target_bir_lowering=False, debug=False,
                   enable_asserts=False, num_devices=N_CORES)

    XT4 = nc.dram_tensor("xt", [N_PC, 128, NE, PC], dt.bfloat16, kind="ExternalInput").ap()
    WQT = nc.dram_tensor("wqt", [128, NE, OSH], dt.bfloat16, kind="ExternalInput").ap()
    WKT = nc.dram_tensor("wkt", [128, NE, D], dt.bfloat16, kind="ExternalInput").ap()
    WVT = nc.dram_tensor("wvt", [128, NE, D], dt.bfloat16, kind="ExternalInput").ap()
    WOT = nc.dram_tensor("wot", [128, NE, OSH], dt.bfloat16, kind="ExternalInput").ap()
    ROPE = nc.dram_tensor("rope", [4, D, S], dt.bfloat16, kind="ExternalInput").ap()
    TRI = nc.dram_tensor("tri", [4, 128, AC], dt.bfloat16, kind="ExternalInput").ap()
    OUT = nc.dram_tensor("out", [S, OSH], dt.float32, kind="ExternalOutput").ap()

    with tile.TileContext(nc) as tc:
        with (
            tc.tile_pool(name="wpool", bufs=1) as wpool,
            tc.tile_pool(name="qkv", bufs=1) as qkvpool,
            tc.tile_pool(name="consts", bufs=1) as cpool,
            tc.tile_pool(name="epool", bufs=3) as epool,
            tc.tile_pool(name="small", bufs=2) as smallpool,
            tc.tile_pool(name="attn", bufs=2) as attnpool,
            tc.tile_pool(name="outs", bufs=2) as outpool,
            tc.tile_pool(name="ps_qkv", bufs=2, space="PSUM") as ps_qkv,
            tc.tile_pool(name="ps_s", bufs=3, space="PSUM") as ps_s,
            tc.tile_pool(name="ps_ot", bufs=3, space="PSUM") as ps_ot,
            tc.tile_pool(name="dram", bufs=3, space="DRAM") as drampool,
        ):
            # phase-1-only pools; closed after phase 1 so the o_proj input
            # pool (agp) can reuse their SBUF space
            p1ctx = ExitStack()
            xspool = p1ctx.enter_context(tc.tile_pool(name="xs", bufs=2))
            cspool = p1ctx.enter_context(tc.tile_pool(name="cs", bufs=2))
            rtmppool = p1ctx.enter_context(tc.tile_pool(name="rtmp", bufs=1))

            # --- resident weights, [128, NE, width]. Priority order: the
            # first chunk's activations and Wk jump ahead of the big weight
            # burst so the PE starts in ~10us instead of ~70.
            wq_sb = wpool.tile([128, NE, OSH], dt.bfloat16)
            wk_sb = wpool.tile([128, NE, D], dt.bfloat16)
            wv_sb = wpool.tile([128, NE, D], dt.bfloat16)
            wo_sb = wpool.tile([128, NE, OSH], dt.bfloat16)

            xs0 = xspool.tile([128, NE, PC], dt.bfloat16, tag="xs")
            cs0 = cspool.tile([128, 4, PC], dt.bfloat16, tag="cs")
            for q in range(8):
                nc.sync.dma_start(xs0[:, q * 4:(q + 1) * 4, :],
                                  XT4[0, :, q * 4:(q + 1) * 4, :])
            for q in range(8):
                nc.sync.dma_start(wk_sb[:, q * 4:(q + 1) * 4, :],
                                  WKT[:, q * 4:(q + 1) * 4, :])
            for q in range(8):
                nc.sync.dma_start(wv_sb[:, q * 4:(q + 1) * 4, :],
                                  WVT[:, q * 4:(q + 1) * 4, :])
            nc.sync.dma_start(cs0[:], ROPE[:, :, 0:PC].rearrange("j p s -> p j s"))
            for q in range(8):
                nc.sync.dma_start(wq_sb[:, q * 4:(q + 1) * 4, :],
                                  WQT[:, q * 4:(q + 1) * 4, :])

            tri_sb = cpool.tile([128, 4, AC], dt.bfloat16)
            nc.sync.dma_start(tri_sb[:], TRI.rearrange("j p f -> p j f"))
            ones_col = cpool.tile([128, 1], dt.bfloat16)
            nc.vector.memset(ones_col[:], 1.0)
            ones_row = cpool.tile([1, 128], dt.bfloat16)
            nc.vector.memset(ones_row[:], 1.0)
            # tiny AllGather to absorb first-collective latency, overlapped
            # with phase 1 (collectives run on TOPSP/SDMA, not the engines)
            warm_in = drampool.tile([128, 1], dt.bfloat16, tag="warm_in")
            nc.sync.dma_start(warm_in[:], ones_col[:])
            warm_out = drampool.tile([128 * N_CORES, 1], dt.bfloat16,
                                     tag="warm_out", addr_space="Shared")
            nc.gpsimd.collective_compute(
                "AllGather", mybir.AluOpType.bypass,
                replica_groups=[list(range(N_CORES))],
                ins=[warm_in.opt()], outs=[warm_out.opt()])

            # --- persistent activations
            qT_sb = qkvpool.tile([128, G, S], dt.bfloat16)     # per-head [d, s]
            kT_sb = qkvpool.tile([128, S], dt.bfloat16)        # [d, s]
            v_sb = qkvpool.tile([128, N_ST, D], dt.bfloat16)   # per s-tile [t, d]

            # ================= Phase 1: QKV projections + RoPE ==============
            for sc in range(N_PC):
                s0 = sc * PC
                if sc == 0:
                    xs, cs = xs0, cs0
                else:
                    xs = xspool.tile([128, NE, PC], dt.bfloat16, tag="xs")
                    for q in range(4):
                        nc.sync.dma_start(xs[:, q * 8:(q + 1) * 8, :],
                                          XT4[sc, :, q * 8:(q + 1) * 8, :])
                    cs = cspool.tile([128, 4, PC], dt.bfloat16, tag="cs")
                    nc.sync.dma_start(cs[:], ROPE[:, :, s0:s0 + PC]
                                      .rearrange("j p s -> p j s"))
                cosq_t, sinq_t = cs[:, 0, :], cs[:, 1, :]
                cosk_t, sink_t = cs[:, 2, :], cs[:, 3, :]

                # k head first, then v, then q heads (k/v weights land first)
                for hh in [G, -1] + list(range(G)):
                    if hh == -1:
                        for u in range(PC // 128):
                            st = (s0 // 128) + u
                            pv = ps_qkv.tile([128, D], dt.float32, tag="pq")
                            for e in range(NE):
                                nc.tensor.matmul(pv[:],
                                                 xs[:, e, u * 128:(u + 1) * 128],
                                                 wv_sb[:, e, :],
                                                 start=(e == 0), stop=(e == NE - 1))
                            nc.vector.tensor_copy(v_sb[:, st, :], pv[:])
                        continue
                    is_k = hh == G
                    pq = ps_qkv.tile([128, PC], dt.float32, tag="pq")
                    for e in range(NE):
                        lhsT = (wk_sb[:, e, :] if is_k
                                else wq_sb[:, e, hh * D:(hh + 1) * D])
                        nc.tensor.matmul(pq[:], lhsT, xs[:, e, :],
                                         start=(e == 0), stop=(e == NE - 1))
                    cos_t, sin_t = (cosk_t, sink_t) if is_k else (cosq_t, sinq_t)
                    dest = kT_sb[:, s0:s0 + PC] if is_k \
                        else qT_sb[:, hh, s0:s0 + PC]
                    rt = rtmppool.tile([128, 2, PC], dt.float32, tag="rt")
                    t1, t2 = rt[:, 0, :], rt[:, 1, :]
                    # low half: q'= q_lo*cos_lo + q_hi*sin_mod_lo
                    nc.vector.tensor_mul(t1[0:64, :], pq[0:64, :], cos_t[0:64, :])
                    nc.vector.tensor_mul(t2[0:64, :], pq[64:128, :], sin_t[0:64, :])
                    nc.vector.tensor_add(dest[0:64, :], t1[0:64, :], t2[0:64, :])
                    # high half: q'= q_hi*cos_hi + q_lo*sin_mod_hi
                    nc.vector.tensor_mul(t1[64:128, :], pq[64:128, :], cos_t[64:128, :])
                    nc.vector.tensor_mul(t2[64:128, :], pq[0:64, :], sin_t[64:128, :])
                    nc.vector.tensor_add(dest[64:128, :], t1[64:128, :], t2[64:128, :])


            p1ctx.close()
            p3ctx = ExitStack()
            agpool = p3ctx.enter_context(tc.tile_pool(name="agp", bufs=2))

            # Wo shard load deferred to here so it doesn't fight startup DMAs
            for q in range(2):
                nc.sync.dma_start(wo_sb[:, q * 16:(q + 1) * 16, :],
                                  WOT[:, q * 16:(q + 1) * 16, :])

            # ============ Phase 2: attention as (col-range, head) units with a
            # depth-2 software-pipelined normalize/store tail (PE never waits
            # on the DVE/ACT chain). Chunks 0 and 1 are interleaved so tiny
            # early units still give the tail chain enough runway; the last
            # chunk is split into two half-width AllGathers so the final
            # o_proj pipelines against the last collective instead of
            # serializing behind it.
            # Unit descriptor: (key, c0, cw, nb, diag0, h)
            ag_ins = {}
            ag_outs = {}
            ag_meta = {}            # key -> (c0, cw)
            pending = []

            def make_units():
                units, keys = [], []
                if not causal:
                    for j in range(N_AC):
                        key = str(j)
                        keys.append((key, j * AC, AC, N_ST, N_ST))
                        for h in range(G):
                            units.append((key, j * AC, AC, N_ST, N_ST, h))
                    return units, keys
                # chunks 0,1 interleaved
                for j in (0, 1):
                    keys.append((str(j), j * AC, AC, 4 * (j + 1), 4 * j))
                for h in range(G):
                    for j in (0, 1):
                        units.append((str(j), j * AC, AC, 4 * (j + 1), 4 * j, h))
                keys.append(("2", 2 * AC, AC, 12, 8))
                units += [("2", 2 * AC, AC, 12, 8, h) for h in range(G)]
                for half in (0, 1):
                    c0 = 3 * AC + half * (AC // 2)
                    nb = 14 + 2 * half
                    key = "3a" if half == 0 else "3b"
                    keys.append((key, c0, AC // 2, nb, nb - 2))
                    units += [(key, c0, AC // 2, nb, nb - 2, h) for h in range(G)]
                return units, keys

            units, keys = make_units()
            remaining = {k[0]: G for k in keys}
            for key, c0, cw, nb, d0 in keys:
                ag_meta[key] = (c0, cw)
                ag_ins[key] = drampool.tile([OSH, cw], dt.bfloat16, tag="ag_in",
                                            name=f"ag_in{key}")

            def emit_tail(u):
                key, c0, cw, nb, d0, h, ot, esum = u
                den = ps_s.tile([1, cw], dt.float32, tag="st")
                nc.tensor.matmul(den[:], ones_col[:], esum[:],
                                 start=True, stop=True)
                recip = smallpool.tile([1, cw], dt.bfloat16, tag="recip")
                with nc.allow_low_precision(reason="softmax denom recip bf16"):
                    nc.vector.reciprocal(recip[:], den[:])
                bc_ps = ps_s.tile([128, cw], dt.float32, tag="st")
                nc.tensor.matmul(bc_ps[:], ones_row[:], recip[:],
                                 start=True, stop=True)
                bc_sb = smallpool.tile([128, cw], dt.bfloat16, tag="bc")
                nc.scalar.copy(bc_sb[:], bc_ps[:])
                attnT = attnpool.tile([128, cw], dt.bfloat16, tag="attnT")
                nc.vector.tensor_mul(attnT[:], ot[:], bc_sb[:])
                nc.sync.dma_start(ag_ins[key][h * 128:(h + 1) * 128, :], attnT[:])
                remaining[key] -= 1
                if remaining[key] == 0:
                    ag_out = drampool.tile([E, cw], dt.bfloat16, tag="ag_out",
                                           addr_space="Shared",
                                           name=f"ag_out{key}")
                    nc.gpsimd.collective_compute(
                        "AllGather", mybir.AluOpType.bypass,
                        replica_groups=[list(range(N_CORES))],
                        ins=[ag_ins[key].opt()], outs=[ag_out.opt()])
                    ag_outs[key] = ag_out

            def emit_phase3(key):
                ag_out = ag_outs[key]
                c0, cw = ag_meta[key]
                for hf in range(cw // 256):
                    f0 = hf * 256
                    ag_sb = agpool.tile([128, NE, 256], dt.bfloat16, tag="ag")
                    for eg in range(8):
                        nc.sync.dma_start(
                            ag_sb[:, eg * 4:(eg + 1) * 4, :],
                            ag_out[eg * 512:(eg + 1) * 512, f0:f0 + 256]
                            .rearrange("(n p) s -> p n s", p=128))
                    for t in range(2):
                        po = ps_qkv.tile([128, OSH], dt.float32, tag="pq")
                        for e in range(NE):
                            nc.tensor.matmul(
                                po[:], ag_sb[:, e, t * 128:(t + 1) * 128],
                                wo_sb[:, e, :],
                                start=(e == 0), stop=(e == NE - 1))
                        o_sb = outpool.tile([128, OSH], dt.float32, tag="o")
                        nc.vector.tensor_copy(o_sb[:], po[:])
                        nc.sync.dma_start(
                            OUT[c0 + f0 + t * 128: c0 + f0 + (t + 1) * 128, :],
                            o_sb[:])

            phase3_after = {}        # emit phase3(key) after unit index i
            if causal:
                phase3_after = {13: "0", 17: "1"}   # mid chunk 3a / mid 3b
            else:
                phase3_after = {11: "0", 15: "1"}

            for i, (key, c0, cw, nb, d0, h) in enumerate(units):
                ot = ps_ot.tile([128, cw], dt.float32, tag="ot")
                esum = epool.tile([128, cw], dt.bfloat16, tag="esum")
                for b in range(nb):
                    st_ps = ps_s.tile([128, cw], dt.float32, tag="st")
                    nc.tensor.matmul(st_ps[:], kT_sb[:, b * 128:(b + 1) * 128],
                                     qT_sb[:, h, c0:c0 + cw],
                                     start=True, stop=True)
                    e_sb = epool.tile([128, cw], dt.bfloat16, tag="e")
                    nc.scalar.activation(e_sb[:], st_ps[:],
                                         mybir.ActivationFunctionType.Exp)
                    if causal and b >= d0:
                        nc.vector.tensor_mul(e_sb[:], e_sb[:],
                                             tri_sb[:, b - d0, 0:cw])
                    nc.tensor.matmul(ot[:], v_sb[:, b, :], e_sb[:],
                                     start=(b == 0), stop=(b == nb - 1))
                    with nc.allow_low_precision(reason="softmax denom bf16"):
                        if b == 0:
                            nc.vector.tensor_copy(esum[:], e_sb[:])
                        else:
                            nc.vector.tensor_add(esum[:], esum[:], e_sb[:])
                pending.append((key, c0, cw, nb, d0, h, ot, esum))
                if len(pending) > 2:
                    emit_tail(pending.pop(0))
                if i in phase3_after:
                    emit_phase3(phase3_after[i])

            for u in pending:
                emit_tail(u)
            if causal:
                for key in ("2", "3a", "3b"):
                    emit_phase3(key)
            else:
                for key in ("2", "3"):
                    emit_phase3(key)
            p3ctx.close()

    nc.compile()
    _BUILD_CACHE[key] = nc
    return nc


def _prep_inputs(hidden_states, attention_mask, cos, sin, Wq, Wk, Wv, Wo):
    X = np.asarray(hidden_states, dtype=np.float32).reshape(S, E)
    # [N_PC, 128, NE, PC]: exact SBUF tile layout per chunk -> long DMA runs
    XT4 = np.ascontiguousarray(
        X.reshape(N_PC, PC, NE, 128).transpose(0, 3, 2, 1)).astype(BF16)

    m = np.asarray(attention_mask, dtype=np.float32).reshape(S, S)
    il, ju = np.tril_indices(S), np.triu_indices(S, 1)
    causal = bool(np.all(m[il] == 0.0) and np.all(m[ju] <= -1e8))
    dense = bool(np.all(m == 0.0))
    if not (causal or dense):
        raise NotImplementedError("only causal or all-zero masks supported")

    scale = 1.0 / math.sqrt(D)
    cosT = np.ascontiguousarray(np.asarray(cos, np.float32).reshape(S, D).T)
    sinT = np.ascontiguousarray(np.asarray(sin, np.float32).reshape(S, D).T)
    sin_mod = sinT.copy()
    sin_mod[0:64] *= -1.0
    rope_t = np.stack([cosT * scale, sin_mod * scale, cosT, sin_mod]) \
        .astype(BF16)

    p = np.arange(128)[:, None]
    f = np.arange(AC)[None, :]
    tri = np.stack([(128 * jj + p <= f) for jj in range(4)]).astype(BF16)

    Wq = np.asarray(Wq, np.float32)
    Wk = np.asarray(Wk, np.float32)
    Wv = np.asarray(Wv, np.float32)
    Wo = np.asarray(Wo, np.float32)

    def wtile(Wshard):
        # [out, E] -> SBUF layout [128, NE, out]
        return np.ascontiguousarray(
            Wshard.T.reshape(NE, 128, Wshard.shape[0]).transpose(1, 0, 2)
        ).astype(BF16)

    in_maps = []
    for c in range(N_CORES):
        in_maps.append({
            "xt": XT4,
            "wqt": wtile(Wq[c * OSH:(c + 1) * OSH, :]),
            "wkt": wtile(Wk[c * D:(c + 1) * D, :]),
            "wvt": wtile(Wv[c * D:(c + 1) * D, :]),
            "wot": wtile(Wo[c * OSH:(c + 1) * OSH, :]),
            "rope": rope_t,
            "tri": tri,
        })
    return in_maps, causal


def kernel(hidden_states, attention_mask, cos, sin, Wq, Wk, Wv, Wo,
           _trace=False, _tmpdir=None):
    in_maps, causal = _prep_inputs(hidden_states, attention_mask, cos, sin,
                                   Wq, Wk, Wv, Wo)
    nc = build_bass(causal)
    res = run_bass_kernel_spmd(nc, in_maps, core_ids=list(range(N_CORES)),
                               trace=_trace, tmpdir=_tmpdir)
    out = np.concatenate([res.results[c]["out"] for c in range(N_CORES)], axis=1)
    kernel._last_result = res
    return out.reshape(1, S, E).astype(np.float32)
